# revision 3
# baseline (speedup 1.0000x reference)
"""GCN (3x GCNConv + segment-sum pooling + MLP + log_softmax over graphs)
on 8 Trainium2 NeuronCores.

Strategy: edges sharded by dst range across the 8 cores (graph/data parallel
per the sharding hint). Per conv layer (one SPMD launch): every core
redundantly computes h' = dinv * (h @ W) for all nodes (cheap, avoids
collectives), then processes its own dst-shard's edges with
dma_gather (256B rows) + dma_scatter_add into a per-dst accumulator.
Scatter-add duplicates race on HW, so edges are organized into rounds of
unique dst rows (2 virtual accumulator slots per dst halve the round count;
slots are summed on-chip at finalize). Self-loops are regular edges. The
symmetric norm factorizes: msg = h'[src], out = dinv*(sum msgs) + b.
Pooling = PE matmul with host-built one-hot graph-assignment tiles, emitted
transposed so the final MLP + log_softmax (over the graph axis = free dim)
needs no transposes. Host only does index prep / sharding / reassembly.
"""
import sys
sys.path.insert(0, '/opt/trn_rl_repo')
import numpy as np
from concourse import bass, mybir, bacc, tile
from concourse.bass_utils import run_bass_kernel_spmd

F32 = mybir.dt.float32
I16 = mybir.dt.int16

NC = 8
N = 100000
G = 512
NPER = N // NC              # 12500 dst nodes per core
DT = 98                     # dst tiles of 128 (12544)
NPAD = 100224               # 783 * 128, = 4 * 25056
BUCKET = 25056              # gather bucket rows (int16-safe)
NB = 4
ACC_ROWS = 25600            # >= 2*12544, plus trash
TRASH = 25500               # scatter trash row
CHUNK = 6144                # max idxs per gather/scatter instruction


def _wrap16(ix):
    """[n] -> [128, n//16] int16 (16-partition wrap, replicated to 8 Q7 cores)."""
    w = ix.reshape(-1, 16).T.astype(np.int16)
    return np.tile(w, (8, 1))


def _prep_edges(src, dst):
    """Build per-core chunked gather/scatter index streams.

    Returns (chunk_plan, gflats, sflats):
      chunk_plan: list of (bucket, n) with n % 128 == 0, n <= CHUNK
      gflats[c], sflats[c]: flat int16 arrays; chunk i's indices live at
        [off_i : off_i + 128*(n_i//16)] in partition-major wrapped layout.
    """
    per_core = []   # per core: dict (r,b) -> (gi_local array, v array)
    max_rb = {}
    for c in range(NC):
        m = (dst >= c * NPER) & (dst < (c + 1) * NPER)
        s = src[m].astype(np.int64)
        d = (dst[m] - c * NPER).astype(np.int64)
        # append self-loops (message h'[global own node] -> dst d)
        s = np.concatenate([s, c * NPER + np.arange(NPER, dtype=np.int64)])
        d = np.concatenate([d, np.arange(NPER, dtype=np.int64)])
        o = np.argsort(d, kind='stable')
        s, d = s[o], d[o]
        counts = np.bincount(d, minlength=NPER)
        starts = np.cumsum(counts) - counts
        k = np.arange(len(d)) - np.repeat(starts, counts)
        v = 2 * d + (k & 1)
        r = k >> 1
        b = s // BUCKET
        gi = s - b * BUCKET
        key = r * NB + b
        o2 = np.argsort(key, kind='stable')
        key, gi, v = key[o2], gi[o2], v[o2]
        uk, idx0, cnt = np.unique(key, return_index=True, return_counts=True)
        groups = {}
        for u, i0, n in zip(uk, idx0, cnt):
            groups[int(u)] = (gi[i0:i0 + n], v[i0:i0 + n])
            max_rb[int(u)] = max(max_rb.get(int(u), 0), int(n))
        per_core.append(groups)

    # uniform chunk plan (max over cores), split at CHUNK, pad to x128
    chunk_plan = []   # (bucket, n)
    group_chunks = []  # (key, [n0, n1, ...])
    for u in sorted(max_rb):
        n = max_rb[u]
        ns = []
        while n > 0:
            take = min(CHUNK, n)
            ns.append(-(-take // 128) * 128)
            n -= take
        group_chunks.append((u, ns))
        for np_ in ns:
            chunk_plan.append((u % NB, np_))

    gflats, sflats = [], []
    for c in range(NC):
        gparts, sparts = [], []
        for u, ns in group_chunks:
            gi, v = per_core[c].get(u, (np.zeros(0, np.int64), np.zeros(0, np.int64)))
            tot = sum(ns)
            gi_p = np.zeros(tot, np.int64)
            v_p = np.full(tot, TRASH, np.int64)
            gi_p[:len(gi)] = gi
            v_p[:len(v)] = v
            pos = 0
            for np_ in ns:
                gparts.append(_wrap16(gi_p[pos:pos + np_]).ravel())
                sparts.append(_wrap16(v_p[pos:pos + np_]).ravel())
                pos += np_
        gflats.append(np.concatenate(gparts))
        sflats.append(np.concatenate(sparts))
    return chunk_plan, gflats, sflats


def _build_conv(chunk_plan, L, relu, pool):
    nc = bacc.Bacc(None, target_bir_lowering=False, num_swdge_queues=4)
    hT = nc.declare_dram_parameter("hT", [64, NPAD], F32, isOutput=False)
    Wp = nc.declare_dram_parameter("W", [64, 64], F32, isOutput=False)
    bb = nc.declare_dram_parameter("bb", [128, 64], F32, isOutput=False)
    dinv_full = nc.declare_dram_parameter("dinv_full", [NPAD], F32, isOutput=False)
    dinv_own = nc.declare_dram_parameter("dinv_own", [DT * 128], F32, isOutput=False)
    gflat = nc.declare_dram_parameter("gflat", [L], I16, isOutput=False)
    sflat = nc.declare_dram_parameter("sflat", [L], I16, isOutput=False)
    if pool:
        Pw = nc.declare_dram_parameter("Pw", [DT, 128, 128], F32, isOutput=False)
        pooledT = nc.declare_dram_parameter("pooledT", [64, 128], F32, isOutput=True)
    out_own = nc.declare_dram_parameter("out_own", [DT * 128, 64], F32, isOutput=True)
    hp = nc.dram_tensor("hp", [NPAD, 64], F32)
    acc2 = nc.dram_tensor("acc2", [ACC_ROWS, 64], F32)

    with tile.TileContext(nc) as tc:
        with (
            tc.tile_pool(name="const", bufs=1) as cpool,
            tc.tile_pool(name="lhs", bufs=4) as lpool,
            tc.tile_pool(name="ps", bufs=4, space="PSUM") as pspool,
            tc.tile_pool(name="hv", bufs=4) as hpool,
            tc.tile_pool(name="sc", bufs=4) as scpool,
            tc.tile_pool(name="gi", bufs=8) as gipool,
            tc.tile_pool(name="ms", bufs=6) as mpool,
            tc.tile_pool(name="fin", bufs=4) as fpool,
            tc.tile_pool(name="pp", bufs=1, space="PSUM") as pppool,
        ):
            W_sb = cpool.tile([64, 64], F32, tag="w")
            nc.sync.dma_start(out=W_sb[:], in_=Wp[:, :])
            bb_sb = cpool.tile([128, 64], F32, tag="bb")
            nc.sync.dma_start(out=bb_sb[:], in_=bb[:, :])

            # zero the accumulator (layout-agnostic: all zeros)
            zt = cpool.tile([128, 3200], F32, tag="zt")
            nc.vector.memset(zt[:], 0.0)
            for i in range(4):
                nc.sync.dma_start(
                    out=acc2[i * 6400:(i + 1) * 6400, :].rearrange(
                        "(p a) f -> p (a f)", p=128),
                    in_=zt[:],
                )

            # h' = dinv * (h @ W) for all nodes (redundant on every core)
            for t in range(NPAD // 128):
                lt = lpool.tile([64, 128], F32, tag="lt")
                nc.sync.dma_start(out=lt[:], in_=hT[:, t * 128:(t + 1) * 128])
                ps = pspool.tile([128, 64], F32, tag="ps")
                nc.tensor.matmul(ps[:], lt[:], W_sb[:], start=True, stop=True)
                dt = scpool.tile([128, 1], F32, tag="dt")
                nc.sync.dma_start(
                    out=dt[:],
                    in_=dinv_full[t * 128:(t + 1) * 128].rearrange("(p o) -> p o", o=1))
                ht = hpool.tile([128, 64], F32, tag="ht")
                nc.vector.tensor_scalar_mul(ht[:], ps[:], dt[:])
                nc.sync.dma_start(out=hp[t * 128:(t + 1) * 128, :], in_=ht[:])

            # edge phase: gather h'[src] -> scatter-add into acc2[2*d + slot]
            off = 0
            for i, (bkt, n) in enumerate(chunk_plan):
                w = n // 16
                gi = gipool.tile([128, w], I16, tag="gi")
                nc.sync.dma_start(
                    out=gi[:],
                    in_=gflat[off:off + 128 * w].rearrange("(p w) -> p w", p=128))
                si = gipool.tile([128, w], I16, tag="si")
                nc.sync.dma_start(
                    out=si[:],
                    in_=sflat[off:off + 128 * w].rearrange("(p w) -> p w", p=128))
                off += 128 * w
                ms = mpool.tile([128, n // 128, 64], F32, tag="ms")
                nc.gpsimd.dma_gather(
                    out_ap=ms[:], in_ap=hp[bkt * BUCKET:(bkt + 1) * BUCKET, :],
                    idxs_ap=gi[:], num_idxs=n, num_idxs_reg=n, elem_size=64,
                    single_packet=False, queue_num=i % 4)
                nc.gpsimd.dma_scatter_add(
                    out_ap=acc2[:, :], in_ap=ms[:], idxs_ap=si[:],
                    num_idxs=n, num_idxs_reg=n, elem_size=64,
                    single_packet=False, queue_num=i % 4)

            # finalize own dst slice: out = [relu](dinv*(slot0+slot1) + b)
            for t in range(DT):
                at = fpool.tile([128, 2, 64], F32, tag="at")
                nc.sync.dma_start(
                    out=at[:],
                    in_=acc2[t * 256:(t + 1) * 256, :].rearrange(
                        "(p s) f -> p s f", p=128))
                dt = scpool.tile([128, 1], F32, tag="dto")
                nc.sync.dma_start(
                    out=dt[:],
                    in_=dinv_own[t * 128:(t + 1) * 128].rearrange("(p o) -> p o", o=1))
                ot = fpool.tile([128, 64], F32, tag="ot")
                nc.vector.tensor_add(ot[:], at[:, 0, :], at[:, 1, :])
                nc.vector.tensor_scalar_mul(ot[:], ot[:], dt[:])
                nc.vector.tensor_add(ot[:], ot[:], bb_sb[:])
                if relu:
                    nc.scalar.activation(ot[:], ot[:], mybir.ActivationFunctionType.Relu)
                nc.sync.dma_start(out=out_own[t * 128:(t + 1) * 128, :], in_=ot[:])
                if pool:
                    pt = lpool.tile([128, 128], F32, tag="pt")
                    nc.sync.dma_start(out=pt[:], in_=Pw[t])
                    pp = pppool.tile([64, 128], F32, tag="pp")
                    nc.tensor.matmul(pp[:], ot[:], pt[:],
                                     start=(t == 0), stop=(t == DT - 1))
            if pool:
                pc = hpool.tile([64, 128], F32, tag="pc")
                nc.vector.tensor_copy(pc[:], pp[:])
                nc.sync.dma_start(out=pooledT[:, :], in_=pc[:])
    nc.finalize()
    return nc


def _build_final():
    nc = bacc.Bacc(None, target_bir_lowering=False)
    parts = nc.declare_dram_parameter("parts", [NC, 64, 512], F32, isOutput=False)
    lW1 = nc.declare_dram_parameter("lW1", [64, 32], F32, isOutput=False)
    lb1 = nc.declare_dram_parameter("lb1", [32, 1], F32, isOutput=False)
    lW2 = nc.declare_dram_parameter("lW2", [32, 5], F32, isOutput=False)
    lb2 = nc.declare_dram_parameter("lb2", [5, 1], F32, isOutput=False)
    out = nc.declare_dram_parameter("out", [5, 512], F32, isOutput=True)
    A = mybir.ActivationFunctionType
    with tile.TileContext(nc) as tc:
        with (
            tc.tile_pool(name="sb", bufs=2) as sb,
            tc.tile_pool(name="ps", bufs=2, space="PSUM") as ps,
            tc.tile_pool(name="c1", bufs=1) as c1,
        ):
            pooled = c1.tile([64, 512], F32, tag="pooled")
            nc.sync.dma_start(out=pooled[:], in_=parts[0])
            for i in range(1, NC):
                pt = sb.tile([64, 512], F32, tag="pt")
                nc.sync.dma_start(out=pt[:], in_=parts[i])
                nc.vector.tensor_add(pooled[:], pooled[:], pt[:])
            w1 = c1.tile([64, 32], F32, tag="w1")
            nc.sync.dma_start(out=w1[:], in_=lW1[:, :])
            b1 = c1.tile([32, 1], F32, tag="b1")
            nc.sync.dma_start(out=b1[:], in_=lb1[:, :])
            w2 = c1.tile([32, 5], F32, tag="w2")
            nc.sync.dma_start(out=w2[:], in_=lW2[:, :])
            b2 = c1.tile([5, 1], F32, tag="b2")
            nc.sync.dma_start(out=b2[:], in_=lb2[:, :])

            z1p = ps.tile([32, 512], F32, tag="z1p")
            nc.tensor.matmul(z1p[:], w1[:], pooled[:], start=True, stop=True)
            z1 = sb.tile([32, 512], F32, tag="z1")
            nc.scalar.activation(z1[:], z1p[:], A.Relu, bias=b1[:])
            z2p = ps.tile([5, 512], F32, tag="z2p")
            nc.tensor.matmul(z2p[:], w2[:], z1[:], start=True, stop=True)
            z2 = sb.tile([5, 512], F32, tag="z2")
            nc.vector.tensor_scalar(z2[:], z2p[:], b2[:], None, mybir.AluOpType.add)

            mx = sb.tile([5, 1], F32, tag="mx")
            nc.vector.tensor_reduce(mx[:], z2[:], mybir.AxisListType.X,
                                    mybir.AluOpType.max)
            zc = sb.tile([5, 512], F32, tag="zc")
            nc.vector.tensor_scalar(zc[:], z2[:], mx[:], None,
                                    mybir.AluOpType.subtract)
            ex = sb.tile([5, 512], F32, tag="ex")
            nc.scalar.activation(ex[:], zc[:], A.Exp)
            sm = sb.tile([5, 1], F32, tag="sm")
            nc.vector.tensor_reduce(sm[:], ex[:], mybir.AxisListType.X,
                                    mybir.AluOpType.add)
            ls = sb.tile([5, 1], F32, tag="ls")
            nc.scalar.activation(ls[:], sm[:], A.Ln)
            oo = sb.tile([5, 512], F32, tag="oo")
            nc.vector.tensor_scalar(oo[:], zc[:], ls[:], None,
                                    mybir.AluOpType.subtract)
            nc.sync.dma_start(out=out[:, :], in_=oo[:])
    nc.finalize()
    return nc


def kernel(x, edge_index, batch, W1, b1, W2, b2, W3, b3, lW1, lb1, lW2, lb2):
    x = np.asarray(x, np.float32)
    edge_index = np.asarray(edge_index)
    batch = np.asarray(batch).astype(np.int64)
    src, dst = edge_index[0].astype(np.int64), edge_index[1].astype(np.int64)

    deg = (np.bincount(dst, minlength=N) + 1).astype(np.float64)
    dinv = (1.0 / np.sqrt(deg)).astype(np.float32)
    dinv_pad = np.zeros(NPAD, np.float32)
    dinv_pad[:N] = dinv

    chunk_plan, gflats, sflats = _prep_edges(src, dst)
    L = len(gflats[0])

    def pad_w(w, fin):
        w = np.asarray(w, np.float32)
        wp = np.zeros((64, 64), np.float32)
        wp[:fin, :w.shape[1]] = w
        return wp

    def pad_b(b):
        bp = np.zeros(64, np.float32)
        b = np.asarray(b, np.float32)
        bp[:len(b)] = b
        return np.tile(bp, (128, 1))

    conv_r = _build_conv(chunk_plan, L, relu=True, pool=False)
    conv_p = _build_conv(chunk_plan, L, relu=False, pool=True)

    # per-core pooling one-hots + graph window bases
    g0s, Pws = [], []
    for c in range(NC):
        gb = batch[c * NPER:(c + 1) * NPER]
        g0 = int(gb[0])
        g0s.append(g0)
        P = np.zeros((DT, 128, 128), np.float32)
        for t in range(DT):
            for j in range(128):
                node = t * 128 + j
                if node < NPER:
                    col = int(gb[node]) - g0
                    if 0 <= col < 128:
                        P[t, j, col] = 1.0
        Pws.append(P)

    def run_conv(ncc, hT, Wp, bp, pool):
        ins = []
        for c in range(NC):
            m = {
                "hT": hT, "W": Wp, "bb": bp,
                "dinv_full": dinv_pad,
                "dinv_own": np.pad(dinv[c * NPER:(c + 1) * NPER],
                                   (0, DT * 128 - NPER)),
                "gflat": gflats[c], "sflat": sflats[c],
            }
            if pool:
                m["Pw"] = Pws[c]
            ins.append(m)
        res = run_bass_kernel_spmd(ncc, ins, core_ids=list(range(NC)))
        h = np.zeros((NPAD, 64), np.float32)
        for c in range(NC):
            h[c * NPER:(c + 1) * NPER] = res.results[c]["out_own"][:NPER]
        pooledT = None
        if pool:
            pooledT = np.zeros((NC, 64, 512), np.float32)
            for c in range(NC):
                w = min(128, 512 - g0s[c])
                pooledT[c][:, g0s[c]:g0s[c] + w] = res.results[c]["pooledT"][:, :w]
        return h, pooledT

    hT0 = np.zeros((64, NPAD), np.float32)
    hT0[:3, :N] = x.T
    h1, _ = run_conv(conv_r, hT0, pad_w(W1, 3), pad_b(b1), False)
    h2, _ = run_conv(conv_r, h1.T.copy(), pad_w(W2, 32), pad_b(b2), False)
    _, pooledT = run_conv(conv_p, h2.T.copy(), pad_w(W3, 64), pad_b(b3), True)

    final = _build_final()
    fin = {
        "parts": pooledT,
        "lW1": np.asarray(lW1, np.float32),
        "lb1": np.asarray(lb1, np.float32).reshape(32, 1),
        "lW2": np.asarray(lW2, np.float32),
        "lb2": np.asarray(lb2, np.float32).reshape(5, 1),
    }
    res = run_bass_kernel_spmd(final, [fin] * NC, core_ids=list(range(NC)))
    return np.ascontiguousarray(res.results[0]["out"].T).astype(np.float32)


# revision 5
# speedup vs baseline: 13.1799x; 13.1799x over previous
"""GCN (3x GCNConv + segment-sum pooling + MLP + log_softmax over graphs)
on 8 Trainium2 NeuronCores.

Strategy: edges sharded by dst range across the 8 cores (graph/data parallel
per the sharding hint). Per conv layer (one SPMD launch): every core
redundantly computes h' = dinv * (h @ W) for all nodes (cheap, avoids
collectives), then processes its own dst-shard's edges with
dma_gather (256B rows) + dma_scatter_add into a per-dst accumulator.
Scatter-add duplicates race on HW, so edges are organized into rounds of
unique dst rows (2 virtual accumulator slots per dst halve the round count;
slots are summed on-chip at finalize). Self-loops are regular edges. The
symmetric norm factorizes: msg = h'[src], out = dinv*(sum msgs) + b.
Pooling = PE matmul with host-built one-hot graph-assignment tiles, emitted
transposed so the final MLP + log_softmax (over the graph axis = free dim)
needs no transposes. Host only does index prep / sharding / reassembly.
"""
import sys
import time
sys.path.insert(0, '/opt/trn_rl_repo')
import numpy as np
from concourse import bass, mybir, bacc, tile
from concourse.bass_utils import run_bass_kernel_spmd

F32 = mybir.dt.float32
I16 = mybir.dt.int16

NC = 8
N = 100000
G = 512
NPER = N // NC              # 12500 dst nodes per core
DT = 98                     # dst tiles of 128 (12544)
NPAD = 100224               # 783 * 128, = 4 * 25056
BUCKET = 25056              # gather bucket rows (int16-safe)
NB = 4
ACC_ROWS = 25600            # >= 2*12544, plus trash
TRASH = 25500               # scatter trash row
CHUNK = 6144                # max idxs per gather/scatter instruction


def _wrap16(ix):
    """[n] -> [128, n//16] int16 (16-partition wrap, replicated to 8 Q7 cores)."""
    w = ix.reshape(-1, 16).T.astype(np.int16)
    return np.tile(w, (8, 1))


def _prep_edges(src, dst):
    """Build per-core chunked gather/scatter index streams.

    Returns (chunk_plan, gflats, sflats):
      chunk_plan: list of (bucket, n) with n % 128 == 0, n <= CHUNK
      gflats[c], sflats[c]: flat int16 arrays; chunk i's indices live at
        [off_i : off_i + 128*(n_i//16)] in partition-major wrapped layout.
    """
    per_core = []   # per core: dict (r,b) -> (gi_local array, v array)
    max_rb = {}
    for c in range(NC):
        m = (dst >= c * NPER) & (dst < (c + 1) * NPER)
        s = src[m].astype(np.int64)
        d = (dst[m] - c * NPER).astype(np.int64)
        # append self-loops (message h'[global own node] -> dst d)
        s = np.concatenate([s, c * NPER + np.arange(NPER, dtype=np.int64)])
        d = np.concatenate([d, np.arange(NPER, dtype=np.int64)])
        o = np.argsort(d, kind='stable')
        s, d = s[o], d[o]
        counts = np.bincount(d, minlength=NPER)
        starts = np.cumsum(counts) - counts
        k = np.arange(len(d)) - np.repeat(starts, counts)
        v = 2 * d + (k & 1)
        r = k >> 1
        b = s // BUCKET
        gi = s - b * BUCKET
        key = r * NB + b
        o2 = np.argsort(key, kind='stable')
        key, gi, v = key[o2], gi[o2], v[o2]
        uk, idx0, cnt = np.unique(key, return_index=True, return_counts=True)
        groups = {}
        for u, i0, n in zip(uk, idx0, cnt):
            groups[int(u)] = (gi[i0:i0 + n], v[i0:i0 + n])
            max_rb[int(u)] = max(max_rb.get(int(u), 0), int(n))
        per_core.append(groups)

    # uniform chunk plan (max over cores), split at CHUNK, pad to x128
    chunk_plan = []   # (bucket, n)
    group_chunks = []  # (key, [n0, n1, ...])
    for u in sorted(max_rb):
        n = max_rb[u]
        ns = []
        while n > 0:
            take = min(CHUNK, n)
            ns.append(-(-take // 128) * 128)
            n -= take
        group_chunks.append((u, ns))
        for np_ in ns:
            chunk_plan.append((u % NB, np_))

    gflats, sflats = [], []
    for c in range(NC):
        gparts, sparts = [], []
        for u, ns in group_chunks:
            gi, v = per_core[c].get(u, (np.zeros(0, np.int64), np.zeros(0, np.int64)))
            tot = sum(ns)
            gi_p = np.zeros(tot, np.int64)
            v_p = np.full(tot, TRASH, np.int64)
            gi_p[:len(gi)] = gi
            v_p[:len(v)] = v
            pos = 0
            for np_ in ns:
                gparts.append(_wrap16(gi_p[pos:pos + np_]).ravel())
                sparts.append(_wrap16(v_p[pos:pos + np_]).ravel())
                pos += np_
        gflats.append(np.concatenate(gparts))
        sflats.append(np.concatenate(sparts))
    return chunk_plan, gflats, sflats


def _build_conv(chunk_plan, L, relu, pool):
    nc = bacc.Bacc(None, target_bir_lowering=False, num_swdge_queues=4)
    hT = nc.declare_dram_parameter("hT", [64, NPAD], F32, isOutput=False)
    Wp = nc.declare_dram_parameter("W", [64, 64], F32, isOutput=False)
    bb = nc.declare_dram_parameter("bb", [128, 64], F32, isOutput=False)
    dinv_full = nc.declare_dram_parameter("dinv_full", [NPAD], F32, isOutput=False)
    dinv_own = nc.declare_dram_parameter("dinv_own", [DT * 128], F32, isOutput=False)
    gflat = nc.declare_dram_parameter("gflat", [L], I16, isOutput=False)
    sflat = nc.declare_dram_parameter("sflat", [L], I16, isOutput=False)
    if pool:
        Pw = nc.declare_dram_parameter("Pw", [DT, 128, 128], F32, isOutput=False)
        pooledT = nc.declare_dram_parameter("pooledT", [64, 128], F32, isOutput=True)
    out_own = nc.declare_dram_parameter("out_own", [DT * 128, 64], F32, isOutput=True)
    hp = nc.dram_tensor("hp", [NPAD, 64], F32)
    acc2 = nc.dram_tensor("acc2", [ACC_ROWS, 64], F32)

    with tile.TileContext(nc) as tc:
        with (
            tc.tile_pool(name="const", bufs=1) as cpool,
            tc.tile_pool(name="lhs", bufs=4) as lpool,
            tc.tile_pool(name="ps", bufs=4, space="PSUM") as pspool,
            tc.tile_pool(name="hv", bufs=4) as hpool,
            tc.tile_pool(name="sc", bufs=4) as scpool,
            tc.tile_pool(name="gi", bufs=8) as gipool,
            tc.tile_pool(name="ms", bufs=6) as mpool,
            tc.tile_pool(name="fin", bufs=4) as fpool,
            tc.tile_pool(name="pp", bufs=1, space="PSUM") as pppool,
        ):
            W_sb = cpool.tile([64, 64], F32, tag="w")
            nc.sync.dma_start(out=W_sb[:], in_=Wp[:, :])
            bb_sb = cpool.tile([128, 64], F32, tag="bb")
            nc.sync.dma_start(out=bb_sb[:], in_=bb[:, :])

            # zero the accumulator (layout-agnostic: all zeros)
            zt = cpool.tile([128, 3200], F32, tag="zt")
            nc.vector.memset(zt[:], 0.0)
            for i in range(4):
                nc.sync.dma_start(
                    out=acc2[i * 6400:(i + 1) * 6400, :].rearrange(
                        "(p a) f -> p (a f)", p=128),
                    in_=zt[:],
                )

            # h' = dinv * (h @ W) for all nodes (redundant on every core)
            for t in range(NPAD // 128):
                lt = lpool.tile([64, 128], F32, tag="lt")
                nc.sync.dma_start(out=lt[:], in_=hT[:, t * 128:(t + 1) * 128])
                ps = pspool.tile([128, 64], F32, tag="ps")
                nc.tensor.matmul(ps[:], lt[:], W_sb[:], start=True, stop=True)
                dt = scpool.tile([128, 1], F32, tag="dt")
                nc.sync.dma_start(
                    out=dt[:],
                    in_=dinv_full[t * 128:(t + 1) * 128].rearrange("(p o) -> p o", o=1))
                ht = hpool.tile([128, 64], F32, tag="ht")
                nc.vector.tensor_scalar_mul(ht[:], ps[:], dt[:])
                nc.sync.dma_start(out=hp[t * 128:(t + 1) * 128, :], in_=ht[:])

            # edge phase: gather h'[src] -> scatter-add into acc2[2*d + slot]
            off = 0
            for i, (bkt, n) in enumerate(chunk_plan):
                w = n // 16
                gi = gipool.tile([128, w], I16, tag="gi")
                nc.sync.dma_start(
                    out=gi[:],
                    in_=gflat[off:off + 128 * w].rearrange("(p w) -> p w", p=128))
                si = gipool.tile([128, w], I16, tag="si")
                nc.sync.dma_start(
                    out=si[:],
                    in_=sflat[off:off + 128 * w].rearrange("(p w) -> p w", p=128))
                off += 128 * w
                ms = mpool.tile([128, n // 128, 64], F32, tag="ms")
                nc.gpsimd.dma_gather(
                    out_ap=ms[:], in_ap=hp[bkt * BUCKET:(bkt + 1) * BUCKET, :],
                    idxs_ap=gi[:], num_idxs=n, num_idxs_reg=n, elem_size=64,
                    single_packet=False, queue_num=i % 4)
                nc.gpsimd.dma_scatter_add(
                    out_ap=acc2[:, :], in_ap=ms[:], idxs_ap=si[:],
                    num_idxs=n, num_idxs_reg=n, elem_size=64,
                    single_packet=False, queue_num=i % 4)

            # finalize own dst slice: out = [relu](dinv*(slot0+slot1) + b)
            for t in range(DT):
                at = fpool.tile([128, 2, 64], F32, tag="at")
                nc.sync.dma_start(
                    out=at[:],
                    in_=acc2[t * 256:(t + 1) * 256, :].rearrange(
                        "(p s) f -> p s f", p=128))
                dt = scpool.tile([128, 1], F32, tag="dto")
                nc.sync.dma_start(
                    out=dt[:],
                    in_=dinv_own[t * 128:(t + 1) * 128].rearrange("(p o) -> p o", o=1))
                ot = fpool.tile([128, 64], F32, tag="ot")
                nc.vector.tensor_add(ot[:], at[:, 0, :], at[:, 1, :])
                nc.vector.tensor_scalar_mul(ot[:], ot[:], dt[:])
                nc.vector.tensor_add(ot[:], ot[:], bb_sb[:])
                if relu:
                    nc.scalar.activation(ot[:], ot[:], mybir.ActivationFunctionType.Relu)
                nc.sync.dma_start(out=out_own[t * 128:(t + 1) * 128, :], in_=ot[:])
                if pool:
                    pt = lpool.tile([128, 128], F32, tag="pt")
                    nc.sync.dma_start(out=pt[:], in_=Pw[t])
                    pp = pppool.tile([64, 128], F32, tag="pp")
                    nc.tensor.matmul(pp[:], ot[:], pt[:],
                                     start=(t == 0), stop=(t == DT - 1))
            if pool:
                pc = hpool.tile([64, 128], F32, tag="pc")
                nc.vector.tensor_copy(pc[:], pp[:])
                nc.sync.dma_start(out=pooledT[:, :], in_=pc[:])
    nc.finalize()
    return nc


def _build_final():
    nc = bacc.Bacc(None, target_bir_lowering=False)
    parts = nc.declare_dram_parameter("parts", [NC, 64, 512], F32, isOutput=False)
    lW1 = nc.declare_dram_parameter("lW1", [64, 32], F32, isOutput=False)
    lb1 = nc.declare_dram_parameter("lb1", [32, 1], F32, isOutput=False)
    lW2 = nc.declare_dram_parameter("lW2", [32, 5], F32, isOutput=False)
    lb2 = nc.declare_dram_parameter("lb2", [5, 1], F32, isOutput=False)
    out = nc.declare_dram_parameter("out", [5, 512], F32, isOutput=True)
    A = mybir.ActivationFunctionType
    with tile.TileContext(nc) as tc:
        with (
            tc.tile_pool(name="sb", bufs=2) as sb,
            tc.tile_pool(name="ps", bufs=2, space="PSUM") as ps,
            tc.tile_pool(name="c1", bufs=1) as c1,
        ):
            pooled = c1.tile([64, 512], F32, tag="pooled")
            nc.sync.dma_start(out=pooled[:], in_=parts[0])
            for i in range(1, NC):
                pt = sb.tile([64, 512], F32, tag="pt")
                nc.sync.dma_start(out=pt[:], in_=parts[i])
                nc.vector.tensor_add(pooled[:], pooled[:], pt[:])
            w1 = c1.tile([64, 32], F32, tag="w1")
            nc.sync.dma_start(out=w1[:], in_=lW1[:, :])
            b1 = c1.tile([32, 1], F32, tag="b1")
            nc.sync.dma_start(out=b1[:], in_=lb1[:, :])
            w2 = c1.tile([32, 5], F32, tag="w2")
            nc.sync.dma_start(out=w2[:], in_=lW2[:, :])
            b2 = c1.tile([5, 1], F32, tag="b2")
            nc.sync.dma_start(out=b2[:], in_=lb2[:, :])

            z1p = ps.tile([32, 512], F32, tag="z1p")
            nc.tensor.matmul(z1p[:], w1[:], pooled[:], start=True, stop=True)
            z1 = sb.tile([32, 512], F32, tag="z1")
            nc.scalar.activation(z1[:], z1p[:], A.Relu, bias=b1[:])
            z2p = ps.tile([5, 512], F32, tag="z2p")
            nc.tensor.matmul(z2p[:], w2[:], z1[:], start=True, stop=True)
            z2 = sb.tile([5, 512], F32, tag="z2")
            nc.vector.tensor_scalar(z2[:], z2p[:], b2[:], None, mybir.AluOpType.add)

            mx = sb.tile([5, 1], F32, tag="mx")
            nc.vector.tensor_reduce(mx[:], z2[:], mybir.AxisListType.X,
                                    mybir.AluOpType.max)
            zc = sb.tile([5, 512], F32, tag="zc")
            nc.vector.tensor_scalar(zc[:], z2[:], mx[:], None,
                                    mybir.AluOpType.subtract)
            ex = sb.tile([5, 512], F32, tag="ex")
            nc.scalar.activation(ex[:], zc[:], A.Exp)
            sm = sb.tile([5, 1], F32, tag="sm")
            nc.vector.tensor_reduce(sm[:], ex[:], mybir.AxisListType.X,
                                    mybir.AluOpType.add)
            ls = sb.tile([5, 1], F32, tag="ls")
            nc.scalar.activation(ls[:], sm[:], A.Ln)
            oo = sb.tile([5, 512], F32, tag="oo")
            nc.vector.tensor_scalar(oo[:], zc[:], ls[:], None,
                                    mybir.AluOpType.subtract)
            nc.sync.dma_start(out=out[:, :], in_=oo[:])
    nc.finalize()
    return nc


def kernel(x, edge_index, batch, W1, b1, W2, b2, W3, b3, lW1, lb1, lW2, lb2):
    kernel.launch_times = []
    x = np.asarray(x, np.float32)
    edge_index = np.asarray(edge_index)
    batch = np.asarray(batch).astype(np.int64)
    src, dst = edge_index[0].astype(np.int64), edge_index[1].astype(np.int64)

    deg = (np.bincount(dst, minlength=N) + 1).astype(np.float64)
    dinv = (1.0 / np.sqrt(deg)).astype(np.float32)
    dinv_pad = np.zeros(NPAD, np.float32)
    dinv_pad[:N] = dinv

    chunk_plan, gflats, sflats = _prep_edges(src, dst)
    L = len(gflats[0])

    def pad_w(w, fin):
        w = np.asarray(w, np.float32)
        wp = np.zeros((64, 64), np.float32)
        wp[:fin, :w.shape[1]] = w
        return wp

    def pad_b(b):
        bp = np.zeros(64, np.float32)
        b = np.asarray(b, np.float32)
        bp[:len(b)] = b
        return np.tile(bp, (128, 1))

    conv_r = _build_conv(chunk_plan, L, relu=True, pool=False)
    conv_p = _build_conv(chunk_plan, L, relu=False, pool=True)

    # per-core pooling one-hots + graph window bases
    g0s, Pws = [], []
    for c in range(NC):
        gb = batch[c * NPER:(c + 1) * NPER]
        g0 = int(gb[0])
        g0s.append(g0)
        P = np.zeros((DT, 128, 128), np.float32)
        for t in range(DT):
            for j in range(128):
                node = t * 128 + j
                if node < NPER:
                    col = int(gb[node]) - g0
                    if 0 <= col < 128:
                        P[t, j, col] = 1.0
        Pws.append(P)

    def run_conv(ncc, hT, Wp, bp, pool):
        ins = []
        for c in range(NC):
            m = {
                "hT": hT, "W": Wp, "bb": bp,
                "dinv_full": dinv_pad,
                "dinv_own": np.pad(dinv[c * NPER:(c + 1) * NPER],
                                   (0, DT * 128 - NPER)),
                "gflat": gflats[c], "sflat": sflats[c],
            }
            if pool:
                m["Pw"] = Pws[c]
            ins.append(m)
        t0 = time.perf_counter()
        res = run_bass_kernel_spmd(ncc, ins, core_ids=list(range(NC)))
        kernel.launch_times.append(time.perf_counter() - t0)
        h = np.zeros((NPAD, 64), np.float32)
        for c in range(NC):
            h[c * NPER:(c + 1) * NPER] = res.results[c]["out_own"][:NPER]
        pooledT = None
        if pool:
            pooledT = np.zeros((NC, 64, 512), np.float32)
            for c in range(NC):
                w = min(128, 512 - g0s[c])
                pooledT[c][:, g0s[c]:g0s[c] + w] = res.results[c]["pooledT"][:, :w]
        return h, pooledT

    hT0 = np.zeros((64, NPAD), np.float32)
    hT0[:3, :N] = x.T
    h1, _ = run_conv(conv_r, hT0, pad_w(W1, 3), pad_b(b1), False)
    h2, _ = run_conv(conv_r, h1.T.copy(), pad_w(W2, 32), pad_b(b2), False)
    _, pooledT = run_conv(conv_p, h2.T.copy(), pad_w(W3, 64), pad_b(b3), True)

    final = _build_final()
    fin = {
        "parts": pooledT,
        "lW1": np.asarray(lW1, np.float32),
        "lb1": np.asarray(lb1, np.float32).reshape(32, 1),
        "lW2": np.asarray(lW2, np.float32),
        "lb2": np.asarray(lb2, np.float32).reshape(5, 1),
    }
    t0 = time.perf_counter()
    res = run_bass_kernel_spmd(final, [fin] * NC, core_ids=list(range(NC)))
    kernel.launch_times.append(time.perf_counter() - t0)
    return np.ascontiguousarray(res.results[0]["out"].T).astype(np.float32)


# revision 6
# speedup vs baseline: 1597.7875x; 121.2292x over previous
"""GCN (3x GCNConv + segment-sum pooling + MLP + log_softmax over graphs)
on 8 Trainium2 NeuronCores.

Strategy: edges sharded by dst range across the 8 cores (graph/data parallel
per the sharding hint). Per conv layer (one SPMD launch): every core
redundantly computes h' = dinv * (h @ W) for all nodes (cheap, avoids
collectives), then processes its own dst-shard's edges with
dma_gather (256B rows) + dma_scatter_add into a per-dst accumulator.
Scatter-add duplicates race on HW, so edges are organized into rounds of
unique dst rows (2 virtual accumulator slots per dst halve the round count;
slots are summed on-chip at finalize). Self-loops are regular edges. The
symmetric norm factorizes: msg = h'[src], out = dinv*(sum msgs) + b.
Pooling = PE matmul with host-built one-hot graph-assignment tiles, emitted
transposed so the final MLP + log_softmax (over the graph axis = free dim)
needs no transposes. Host only does index prep / sharding / reassembly.
"""
import sys
import time
sys.path.insert(0, '/opt/trn_rl_repo')
import numpy as np
from concourse import bass, mybir, bacc, tile
from concourse.bass_utils import run_bass_kernel_spmd

F32 = mybir.dt.float32
I16 = mybir.dt.int16

NC = 8
N = 100000
G = 512
NPER = N // NC              # 12500 dst nodes per core
DT = 98                     # dst tiles of 128 (12544)
NPAD = 100224               # 783 * 128, = 4 * 25056
BUCKET = 25056              # gather bucket rows (int16-safe)
NB = 4
ACC_ROWS = 25600            # >= 2*12544, plus trash
TRASH = 25500               # scatter trash row
CHUNK = 6144                # max idxs per gather/scatter instruction


def _wrap16(ix):
    """[n] -> [128, n//16] int16 (16-partition wrap, replicated to 8 Q7 cores)."""
    w = ix.reshape(-1, 16).T.astype(np.int16)
    return np.tile(w, (8, 1))


def _prep_edges(src, dst):
    """Build per-core chunked gather/scatter index streams.

    Returns (chunk_plan, gflats, sflats):
      chunk_plan: list of (bucket, n) with n % 128 == 0, n <= CHUNK
      gflats[c], sflats[c]: flat int16 arrays; chunk i's indices live at
        [off_i : off_i + 128*(n_i//16)] in partition-major wrapped layout.
    """
    per_core = []   # per core: dict (r,b) -> (gi_local array, v array)
    max_rb = {}
    for c in range(NC):
        m = (dst >= c * NPER) & (dst < (c + 1) * NPER)
        s = src[m].astype(np.int64)
        d = (dst[m] - c * NPER).astype(np.int64)
        # append self-loops (message h'[global own node] -> dst d)
        s = np.concatenate([s, c * NPER + np.arange(NPER, dtype=np.int64)])
        d = np.concatenate([d, np.arange(NPER, dtype=np.int64)])
        o = np.argsort(d, kind='stable')
        s, d = s[o], d[o]
        counts = np.bincount(d, minlength=NPER)
        starts = np.cumsum(counts) - counts
        k = np.arange(len(d)) - np.repeat(starts, counts)
        v = 2 * d + (k & 1)
        r = k >> 1
        b = s // BUCKET
        gi = s - b * BUCKET
        key = r * NB + b
        o2 = np.argsort(key, kind='stable')
        key, gi, v = key[o2], gi[o2], v[o2]
        uk, idx0, cnt = np.unique(key, return_index=True, return_counts=True)
        groups = {}
        for u, i0, n in zip(uk, idx0, cnt):
            groups[int(u)] = (gi[i0:i0 + n], v[i0:i0 + n])
            max_rb[int(u)] = max(max_rb.get(int(u), 0), int(n))
        per_core.append(groups)

    # uniform chunk plan (max over cores), split at CHUNK, pad to x128
    chunk_plan = []   # (bucket, n)
    group_chunks = []  # (key, [n0, n1, ...])
    for u in sorted(max_rb):
        n = max_rb[u]
        ns = []
        while n > 0:
            take = min(CHUNK, n)
            ns.append(-(-take // 128) * 128)
            n -= take
        group_chunks.append((u, ns))
        for np_ in ns:
            chunk_plan.append((u % NB, np_))

    gflats, sflats = [], []
    for c in range(NC):
        gparts, sparts = [], []
        for u, ns in group_chunks:
            gi, v = per_core[c].get(u, (np.zeros(0, np.int64), np.zeros(0, np.int64)))
            tot = sum(ns)
            gi_p = np.zeros(tot, np.int64)
            v_p = np.full(tot, TRASH, np.int64)
            gi_p[:len(gi)] = gi
            v_p[:len(v)] = v
            pos = 0
            for np_ in ns:
                gparts.append(_wrap16(gi_p[pos:pos + np_]).ravel())
                sparts.append(_wrap16(v_p[pos:pos + np_]).ravel())
                pos += np_
        gflats.append(np.concatenate(gparts))
        sflats.append(np.concatenate(sparts))
    return chunk_plan, gflats, sflats


def _build_conv(chunk_plan, L, relu, pool, skip_edges=False):
    nc = bacc.Bacc(None, target_bir_lowering=False, num_swdge_queues=4)
    hT = nc.declare_dram_parameter("hT", [64, NPAD], F32, isOutput=False)
    Wp = nc.declare_dram_parameter("W", [64, 64], F32, isOutput=False)
    bb = nc.declare_dram_parameter("bb", [128, 64], F32, isOutput=False)
    dinv_full = nc.declare_dram_parameter("dinv_full", [NPAD], F32, isOutput=False)
    dinv_own = nc.declare_dram_parameter("dinv_own", [DT * 128], F32, isOutput=False)
    gflat = nc.declare_dram_parameter("gflat", [L], I16, isOutput=False)
    sflat = nc.declare_dram_parameter("sflat", [L], I16, isOutput=False)
    if pool:
        Pw = nc.declare_dram_parameter("Pw", [DT, 128, 128], F32, isOutput=False)
        pooledT = nc.declare_dram_parameter("pooledT", [64, 128], F32, isOutput=True)
    out_own = nc.declare_dram_parameter("out_own", [DT * 128, 64], F32, isOutput=True)
    hp = nc.dram_tensor("hp", [NPAD, 64], F32)
    acc2 = nc.dram_tensor("acc2", [ACC_ROWS, 64], F32)

    with tile.TileContext(nc) as tc:
        with (
            tc.tile_pool(name="const", bufs=1) as cpool,
            tc.tile_pool(name="lhs", bufs=4) as lpool,
            tc.tile_pool(name="ps", bufs=4, space="PSUM") as pspool,
            tc.tile_pool(name="hv", bufs=4) as hpool,
            tc.tile_pool(name="sc", bufs=4) as scpool,
            tc.tile_pool(name="gi", bufs=8) as gipool,
            tc.tile_pool(name="ms", bufs=6) as mpool,
            tc.tile_pool(name="fin", bufs=4) as fpool,
            tc.tile_pool(name="pp", bufs=1, space="PSUM") as pppool,
        ):
            W_sb = cpool.tile([64, 64], F32, tag="w")
            nc.sync.dma_start(out=W_sb[:], in_=Wp[:, :])
            bb_sb = cpool.tile([128, 64], F32, tag="bb")
            nc.sync.dma_start(out=bb_sb[:], in_=bb[:, :])

            # zero the accumulator (layout-agnostic: all zeros)
            zt = cpool.tile([128, 3200], F32, tag="zt")
            nc.vector.memset(zt[:], 0.0)
            for i in range(4):
                nc.sync.dma_start(
                    out=acc2[i * 6400:(i + 1) * 6400, :].rearrange(
                        "(p a) f -> p (a f)", p=128),
                    in_=zt[:],
                )

            # h' = dinv * (h @ W) for all nodes (redundant on every core)
            for t in range(NPAD // 128):
                lt = lpool.tile([64, 128], F32, tag="lt")
                nc.sync.dma_start(out=lt[:], in_=hT[:, t * 128:(t + 1) * 128])
                ps = pspool.tile([128, 64], F32, tag="ps")
                nc.tensor.matmul(ps[:], lt[:], W_sb[:], start=True, stop=True)
                dt = scpool.tile([128, 1], F32, tag="dt")
                nc.sync.dma_start(
                    out=dt[:],
                    in_=dinv_full[t * 128:(t + 1) * 128].rearrange("(p o) -> p o", o=1))
                ht = hpool.tile([128, 64], F32, tag="ht")
                nc.vector.tensor_scalar_mul(ht[:], ps[:], dt[:])
                nc.sync.dma_start(out=hp[t * 128:(t + 1) * 128, :], in_=ht[:])

            # edge phase: gather h'[src] -> scatter-add into acc2[2*d + slot]
            off = 0
            for i, (bkt, n) in enumerate(chunk_plan if not skip_edges else []):
                w = n // 16
                gi = gipool.tile([128, w], I16, tag="gi")
                nc.sync.dma_start(
                    out=gi[:],
                    in_=gflat[off:off + 128 * w].rearrange("(p w) -> p w", p=128))
                si = gipool.tile([128, w], I16, tag="si")
                nc.sync.dma_start(
                    out=si[:],
                    in_=sflat[off:off + 128 * w].rearrange("(p w) -> p w", p=128))
                off += 128 * w
                ms = mpool.tile([128, n // 128, 64], F32, tag="ms")
                nc.gpsimd.dma_gather(
                    out_ap=ms[:], in_ap=hp[bkt * BUCKET:(bkt + 1) * BUCKET, :],
                    idxs_ap=gi[:], num_idxs=n, num_idxs_reg=n, elem_size=64,
                    single_packet=False, queue_num=i % 4)
                nc.gpsimd.dma_scatter_add(
                    out_ap=acc2[:, :], in_ap=ms[:], idxs_ap=si[:],
                    num_idxs=n, num_idxs_reg=n, elem_size=64,
                    single_packet=False, queue_num=i % 4)

            # finalize own dst slice: out = [relu](dinv*(slot0+slot1) + b)
            for t in range(DT):
                at = fpool.tile([128, 2, 64], F32, tag="at")
                nc.sync.dma_start(
                    out=at[:],
                    in_=acc2[t * 256:(t + 1) * 256, :].rearrange(
                        "(p s) f -> p s f", p=128))
                dt = scpool.tile([128, 1], F32, tag="dto")
                nc.sync.dma_start(
                    out=dt[:],
                    in_=dinv_own[t * 128:(t + 1) * 128].rearrange("(p o) -> p o", o=1))
                ot = fpool.tile([128, 64], F32, tag="ot")
                nc.vector.tensor_add(ot[:], at[:, 0, :], at[:, 1, :])
                nc.vector.tensor_scalar_mul(ot[:], ot[:], dt[:])
                nc.vector.tensor_add(ot[:], ot[:], bb_sb[:])
                if relu:
                    nc.scalar.activation(ot[:], ot[:], mybir.ActivationFunctionType.Relu)
                nc.sync.dma_start(out=out_own[t * 128:(t + 1) * 128, :], in_=ot[:])
                if pool:
                    pt = lpool.tile([128, 128], F32, tag="pt")
                    nc.sync.dma_start(out=pt[:], in_=Pw[t])
                    pp = pppool.tile([64, 128], F32, tag="pp")
                    nc.tensor.matmul(pp[:], ot[:], pt[:],
                                     start=(t == 0), stop=(t == DT - 1))
            if pool:
                pc = hpool.tile([64, 128], F32, tag="pc")
                nc.vector.tensor_copy(pc[:], pp[:])
                nc.sync.dma_start(out=pooledT[:, :], in_=pc[:])
    nc.finalize()
    return nc


def _build_final():
    nc = bacc.Bacc(None, target_bir_lowering=False)
    parts = nc.declare_dram_parameter("parts", [NC, 64, 512], F32, isOutput=False)
    lW1 = nc.declare_dram_parameter("lW1", [64, 32], F32, isOutput=False)
    lb1 = nc.declare_dram_parameter("lb1", [32, 1], F32, isOutput=False)
    lW2 = nc.declare_dram_parameter("lW2", [32, 5], F32, isOutput=False)
    lb2 = nc.declare_dram_parameter("lb2", [5, 1], F32, isOutput=False)
    out = nc.declare_dram_parameter("out", [5, 512], F32, isOutput=True)
    A = mybir.ActivationFunctionType
    with tile.TileContext(nc) as tc:
        with (
            tc.tile_pool(name="sb", bufs=2) as sb,
            tc.tile_pool(name="ps", bufs=2, space="PSUM") as ps,
            tc.tile_pool(name="c1", bufs=1) as c1,
        ):
            pooled = c1.tile([64, 512], F32, tag="pooled")
            nc.sync.dma_start(out=pooled[:], in_=parts[0])
            for i in range(1, NC):
                pt = sb.tile([64, 512], F32, tag="pt")
                nc.sync.dma_start(out=pt[:], in_=parts[i])
                nc.vector.tensor_add(pooled[:], pooled[:], pt[:])
            w1 = c1.tile([64, 32], F32, tag="w1")
            nc.sync.dma_start(out=w1[:], in_=lW1[:, :])
            b1 = c1.tile([32, 1], F32, tag="b1")
            nc.sync.dma_start(out=b1[:], in_=lb1[:, :])
            w2 = c1.tile([32, 5], F32, tag="w2")
            nc.sync.dma_start(out=w2[:], in_=lW2[:, :])
            b2 = c1.tile([5, 1], F32, tag="b2")
            nc.sync.dma_start(out=b2[:], in_=lb2[:, :])

            z1p = ps.tile([32, 512], F32, tag="z1p")
            nc.tensor.matmul(z1p[:], w1[:], pooled[:], start=True, stop=True)
            z1 = sb.tile([32, 512], F32, tag="z1")
            nc.scalar.activation(z1[:], z1p[:], A.Relu, bias=b1[:])
            z2p = ps.tile([5, 512], F32, tag="z2p")
            nc.tensor.matmul(z2p[:], w2[:], z1[:], start=True, stop=True)
            z2 = sb.tile([5, 512], F32, tag="z2")
            nc.vector.tensor_scalar(z2[:], z2p[:], b2[:], None, mybir.AluOpType.add)

            mx = sb.tile([5, 1], F32, tag="mx")
            nc.vector.tensor_reduce(mx[:], z2[:], mybir.AxisListType.X,
                                    mybir.AluOpType.max)
            zc = sb.tile([5, 512], F32, tag="zc")
            nc.vector.tensor_scalar(zc[:], z2[:], mx[:], None,
                                    mybir.AluOpType.subtract)
            ex = sb.tile([5, 512], F32, tag="ex")
            nc.scalar.activation(ex[:], zc[:], A.Exp)
            sm = sb.tile([5, 1], F32, tag="sm")
            nc.vector.tensor_reduce(sm[:], ex[:], mybir.AxisListType.X,
                                    mybir.AluOpType.add)
            ls = sb.tile([5, 1], F32, tag="ls")
            nc.scalar.activation(ls[:], sm[:], A.Ln)
            oo = sb.tile([5, 512], F32, tag="oo")
            nc.vector.tensor_scalar(oo[:], zc[:], ls[:], None,
                                    mybir.AluOpType.subtract)
            nc.sync.dma_start(out=out[:, :], in_=oo[:])
    nc.finalize()
    return nc


def kernel(x, edge_index, batch, W1, b1, W2, b2, W3, b3, lW1, lb1, lW2, lb2):
    kernel.launch_times = []
    x = np.asarray(x, np.float32)
    edge_index = np.asarray(edge_index)
    batch = np.asarray(batch).astype(np.int64)
    src, dst = edge_index[0].astype(np.int64), edge_index[1].astype(np.int64)

    deg = (np.bincount(dst, minlength=N) + 1).astype(np.float64)
    dinv = (1.0 / np.sqrt(deg)).astype(np.float32)
    dinv_pad = np.zeros(NPAD, np.float32)
    dinv_pad[:N] = dinv

    chunk_plan, gflats, sflats = _prep_edges(src, dst)
    L = len(gflats[0])

    def pad_w(w, fin):
        w = np.asarray(w, np.float32)
        wp = np.zeros((64, 64), np.float32)
        wp[:fin, :w.shape[1]] = w
        return wp

    def pad_b(b):
        bp = np.zeros(64, np.float32)
        b = np.asarray(b, np.float32)
        bp[:len(b)] = b
        return np.tile(bp, (128, 1))

    conv_r = _build_conv(chunk_plan, L, relu=True, pool=False)
    conv_p = _build_conv(chunk_plan, L, relu=False, pool=True)

    # per-core pooling one-hots + graph window bases
    g0s, Pws = [], []
    for c in range(NC):
        gb = batch[c * NPER:(c + 1) * NPER]
        g0 = int(gb[0])
        g0s.append(g0)
        P = np.zeros((DT, 128, 128), np.float32)
        for t in range(DT):
            for j in range(128):
                node = t * 128 + j
                if node < NPER:
                    col = int(gb[node]) - g0
                    if 0 <= col < 128:
                        P[t, j, col] = 1.0
        Pws.append(P)

    def run_conv(ncc, hT, Wp, bp, pool):
        ins = []
        for c in range(NC):
            m = {
                "hT": hT, "W": Wp, "bb": bp,
                "dinv_full": dinv_pad,
                "dinv_own": np.pad(dinv[c * NPER:(c + 1) * NPER],
                                   (0, DT * 128 - NPER)),
                "gflat": gflats[c], "sflat": sflats[c],
            }
            if pool:
                m["Pw"] = Pws[c]
            ins.append(m)
        t0 = time.perf_counter()
        res = run_bass_kernel_spmd(ncc, ins, core_ids=list(range(NC)))
        kernel.launch_times.append(time.perf_counter() - t0)
        h = np.zeros((NPAD, 64), np.float32)
        for c in range(NC):
            h[c * NPER:(c + 1) * NPER] = res.results[c]["out_own"][:NPER]
        pooledT = None
        if pool:
            pooledT = np.zeros((NC, 64, 512), np.float32)
            for c in range(NC):
                w = min(128, 512 - g0s[c])
                pooledT[c][:, g0s[c]:g0s[c] + w] = res.results[c]["pooledT"][:, :w]
        return h, pooledT

    hT0 = np.zeros((64, NPAD), np.float32)
    hT0[:3, :N] = x.T
    h1, _ = run_conv(conv_r, hT0, pad_w(W1, 3), pad_b(b1), False)
    h2, _ = run_conv(conv_r, h1.T.copy(), pad_w(W2, 32), pad_b(b2), False)
    _, pooledT = run_conv(conv_p, h2.T.copy(), pad_w(W3, 64), pad_b(b3), True)

    final = _build_final()
    fin = {
        "parts": pooledT,
        "lW1": np.asarray(lW1, np.float32),
        "lb1": np.asarray(lb1, np.float32).reshape(32, 1),
        "lW2": np.asarray(lW2, np.float32),
        "lb2": np.asarray(lb2, np.float32).reshape(5, 1),
    }
    t0 = time.perf_counter()
    res = run_bass_kernel_spmd(final, [fin] * NC, core_ids=list(range(NC)))
    kernel.launch_times.append(time.perf_counter() - t0)
    return np.ascontiguousarray(res.results[0]["out"].T).astype(np.float32)
